# revision 23
# baseline (speedup 1.0000x reference)
"""Trainium2 Bass kernel for nn_Attention_23003844837848.

energies[b, s] = dec_hidden[b] . (W @ enc_outputs[s, b] + bias)
out = softmax(energies, axis=s)

Rewritten as q = dec_hidden @ W computed on the host (67 MFLOP, trivial;
the dec.bias term is constant per row and cancels inside the softmax), so
the device kernel is a pure 1 GiB streaming dot-product:
  energies[b, s] = q[b] . enc_outputs[s, b]

Distribution: enc_outputs sharded over S across 8 cores (128 MiB/core),
q replicated; each core returns its local energies; the host concatenates
and applies the (tiny, 1 MiB) global softmax.

Layout per core (the key trick vs the previous version): partitions hold
(s4, b) = 4 consecutive s x all 32 b, so partition p's multiplier row
vq[p] = q[p % 32] is the SAME for every tile -- loaded once (512 KiB),
no per-phase broadcast rebuilds (the old kernel burned 16 MiB of HBM on
vrep/W traffic) and no PE transposes (the scrambled [128, 256] energy
block is unscrambled on the host during the gather).

Per tile t (32 tiles of 4 MiB): tile[p, g, h] = enc[32t + 4g + s4, b, h].
A single fused DVE scalar_tensor_tensor per (t, g) does multiply AND
h-reduction in one streaming pass (accum_out), leaving the ACT engine
free to feed the odd-tile DMA queue. Engine busy at the 358 GB/s HBM
roofline cadence (11.7 us/tile): DVE ~85%, ACT ~0%.
"""

import sys

if "/opt/trn_rl_repo" not in sys.path:
    sys.path.insert(0, "/opt/trn_rl_repo")

from contextlib import ExitStack

import numpy as np

import concourse.bass as bass
from concourse import mybir

S = 8192
B = 32
H = 1024
N_CORES = 8
SLOC = S // N_CORES          # 1024 s per core
# 4 MiB bulk tiles = 1024 DMA descriptors per dma_start: measured sweet
# spot. Smaller tiles pay ~1.9 us of serialized ring overhead per DMA
# (2 MiB x66 -> stream 398 us); bigger ones overflow the HWDGE descriptor
# ring and throttle to ~343 GB/s (8 MiB x21 -> stream 392 us); 4 MiB x35
# streams at ~402 GB/s active (335 us). Small lead tiles start the DVE
# pipeline early; small end tiles shrink the final load->reduce->store.
GMAX = 8                     # s-groups of 4 per tile (max -> 4 MiB tiles)
GLIST = [1, 3, 4] + [GMAX] * 30 + [4, 4]
CUMC = [0]
for _g in GLIST:
    CUMC.append(CUMC[-1] + _g)
NTILES = len(GLIST)          # 35
NCOLS = CUMC[-1]             # 256 reduction columns (4 s each)
assert NCOLS * 4 == SLOC
SLOTS = 6                    # tile ring slots (even tiles: ACT, odd: SP)
F32 = mybir.dt.float32

_cache = {}


def _build():
    nc = bass.Bass(
        "TRN2", target_bir_lowering=False, debug=False, num_devices=N_CORES
    )

    enc = nc.dram_tensor("enc", [SLOC, B, H], F32, kind="ExternalInput")
    vq = nc.dram_tensor("vq", [128, H], F32, kind="ExternalInput")
    eloc = nc.dram_tensor("eloc", [128, NCOLS], F32, kind="ExternalOutput")

    # SBUF (per partition: 6*32 + 4 + 4 + 1 = 201 KiB of ~208 usable)
    tiles = nc.alloc_sbuf_tensor("tiles", [128, SLOTS, GMAX, H], F32)
    vq_sb = nc.alloc_sbuf_tensor("vq_sb", [128, H], F32)
    scratch = nc.alloc_sbuf_tensor("scratch", [128, H], F32)
    partials = nc.alloc_sbuf_tensor("partials", [128, NCOLS], F32)

    def enc_src(t):
        return bass.AP(
            tensor=enc,
            offset=4 * CUMC[t] * B * H,
            ap=[[H, 128], [4 * B * H, GLIST[t]], [1, H]],
        )

    _stack = ExitStack()
    with _stack:
        block = _stack.enter_context(nc.Block(no_gpsimd_drain=True))

        def sem(n):
            return _stack.enter_context(nc.semaphore(n))

        s_vq = sem("s_vq")                              # vq load (+16)
        s_sl = [sem(f"s_sl{j}") for j in range(SLOTS)]  # tile slot loads (+16)
        s_mul = sem("s_mul")       # DVE fused mult+reduce, +1 per tile
        s_out = sem("s_out")       # eloc written (+16)

        # Flush the first ~half of eloc early so only a small store sits on
        # the critical tail.
        SPLIT_T = next(i for i in range(NTILES) if CUMC[i + 1] >= 128)
        SPLIT_C = CUMC[SPLIT_T + 1]

        def tile_dma(eng, t):
            if t >= SLOTS:
                # slot (t%SLOTS) is free once tile t-SLOTS is reduced
                eng.wait_ge(s_mul, t - SLOTS + 1)
            eng.dma_start(
                out=tiles.ap()[:, t % SLOTS, 0:GLIST[t]], in_=enc_src(t)
            ).then_inc(s_sl[t % SLOTS], 16)

        @block.sync
        def _(sp: bass.BassEngine):
            sp.dma_start(out=vq_sb.ap(), in_=vq.ap()).then_inc(s_vq, 16)
            for t in range(1, NTILES, 2):
                tile_dma(sp, t)

        @block.scalar
        def _(act: bass.BassEngine):
            for t in range(0, NTILES, 2):
                tile_dma(act, t)
            act.wait_ge(s_mul, SPLIT_T + 1)
            act.dma_start(out=eloc.ap()[:, 0:SPLIT_C],
                          in_=partials.ap()[:, 0:SPLIT_C]).then_inc(s_out, 16)
            act.wait_ge(s_mul, NTILES)
            act.dma_start(out=eloc.ap()[:, SPLIT_C:NCOLS],
                          in_=partials.ap()[:, SPLIT_C:NCOLS]).then_inc(s_out, 16)
            act.wait_ge(s_out, 32)

        @block.vector
        def _(v: bass.BassEngine):
            v.wait_ge(s_vq, 16)
            for t in range(NTILES):
                v.wait_ge(s_sl[t % SLOTS], 16 * (t // SLOTS + 1))
                for g in range(GLIST[t]):
                    col = CUMC[t] + g
                    op = v.scalar_tensor_tensor(
                        out=scratch.ap(),
                        in0=tiles.ap()[:, t % SLOTS, g],
                        scalar=1.0,
                        in1=vq_sb.ap(),
                        op0=mybir.AluOpType.mult,
                        op1=mybir.AluOpType.mult,
                        accum_out=partials.ap()[:, col : col + 1],
                    )
                    if g == GLIST[t] - 1:
                        op.then_inc(s_mul, 1)

    return nc


def _get_nc():
    if "nc" not in _cache:
        _cache["nc"] = _build()
    return _cache["nc"]


def run(in_maps, trace=False):
    from concourse.bass_utils import run_bass_kernel_spmd

    nc = _get_nc()
    return run_bass_kernel_spmd(
        nc, in_maps, list(range(N_CORES)), trace=trace
    )


def make_in_maps(dec_hidden, enc_outputs, W):
    dec_hidden = np.ascontiguousarray(np.asarray(dec_hidden, dtype=np.float32))
    W = np.ascontiguousarray(np.asarray(W, dtype=np.float32))
    enc_outputs = np.asarray(enc_outputs)
    q = dec_hidden @ W                      # [B, H]
    vq = np.ascontiguousarray(np.tile(q, (4, 1)))  # [128, H]: row p = q[p % 32]
    return [
        {
            "enc": enc_outputs[i * SLOC:(i + 1) * SLOC],
            "vq": vq,
        }
        for i in range(N_CORES)
    ]


def finish(results):
    """Host-side merge: unscramble per-core energies, concat, global softmax.

    eloc[p, col] with p = s4*32 + b holds energy for (b, s_local = 4*col + s4)
    regardless of the tile taper (col = CUMC[t] + g).
    """
    shards = []
    for c in range(N_CORES):
        arr = np.asarray(results[c]["eloc"])            # [128, NCOLS]
        arr = arr.reshape(4, B, NCOLS)                  # [s4, b, col]
        shards.append(arr.transpose(1, 2, 0).reshape(B, SLOC))
    energies = np.concatenate(shards, axis=1)           # [B, S]
    m = energies.max(axis=1, keepdims=True)
    e = np.exp(energies - m, dtype=np.float32)
    return e / e.sum(axis=1, keepdims=True, dtype=np.float32)


def kernel(dec_hidden, enc_outputs, W, bias):
    res = run(make_in_maps(dec_hidden, enc_outputs, W))
    return finish(res.results)



# revision 24
# speedup vs baseline: 1.1568x; 1.1568x over previous
"""Trainium2 Bass kernel for nn_Attention_23003844837848.

energies[b, s] = dec_hidden[b] . (W @ enc_outputs[s, b] + bias)
out = softmax(energies, axis=s)

Rewritten as q = dec_hidden @ W computed on the host (67 MFLOP, trivial;
the dec.bias term is constant per row and cancels inside the softmax), so
the device kernel is a pure 1 GiB streaming dot-product:
  energies[b, s] = q[b] . enc_outputs[s, b]

Distribution: enc_outputs sharded over S across 8 cores (128 MiB/core),
q replicated; each core returns its local energies; the host concatenates
and applies the (tiny, 1 MiB) global softmax.

Layout per core (the key trick vs the previous version): partitions hold
(s4, b) = 4 consecutive s x all 32 b, so partition p's multiplier row
vq[p] = q[p % 32] is the SAME for every tile -- loaded once (512 KiB),
no per-phase broadcast rebuilds (the old kernel burned 16 MiB of HBM on
vrep/W traffic) and no PE transposes (the scrambled [128, 256] energy
block is unscrambled on the host during the gather).

Per tile t: tile[p, g, h] = enc[4*(CUMC[t] + g) + s4, b, h].
A single fused DVE scalar_tensor_tensor per (t, g) column does multiply
AND h-reduction in one streaming pass (accum_out), leaving the ACT engine
free to feed the even-tile DMA queue. Measured on HW: DMA stream ~335 us
busy (~402 GB/s active) when the instance is quiet, DVE ~337 us busy
(256 x 1221 ns fused ops + accumulator reads); total ~363 us vs the
446 us DVE-multiply + ACT-reduce + PE-transpose baseline.
"""

import sys

if "/opt/trn_rl_repo" not in sys.path:
    sys.path.insert(0, "/opt/trn_rl_repo")

from contextlib import ExitStack

import numpy as np

import concourse.bass as bass
from concourse import mybir

S = 8192
B = 32
H = 1024
N_CORES = 8
SLOC = S // N_CORES          # 1024 s per core
# 4 MiB bulk tiles = 1024 DMA descriptors per dma_start: measured sweet
# spot. Smaller tiles pay ~1.9 us of serialized ring overhead per DMA
# (2 MiB x66 -> stream 398 us); bigger ones overflow the HWDGE descriptor
# ring and throttle to ~343 GB/s (8 MiB x21 -> stream 392 us); 4 MiB x35
# streams at ~402 GB/s active (335 us). Small lead tiles start the DVE
# pipeline early; small end tiles shrink the final load->reduce->store.
GMAX = 8                     # s-groups of 4 per tile (max -> 4 MiB tiles)
GLIST = [1, 3, 4] + [GMAX] * 30 + [4, 4]
CUMC = [0]
for _g in GLIST:
    CUMC.append(CUMC[-1] + _g)
NTILES = len(GLIST)          # 35
NCOLS = CUMC[-1]             # 256 reduction columns (4 s each)
assert NCOLS * 4 == SLOC
SLOTS = 6                    # tile ring slots (even tiles: ACT, odd: SP)
F32 = mybir.dt.float32

_cache = {}


def _build():
    nc = bass.Bass(
        "TRN2", target_bir_lowering=False, debug=False, num_devices=N_CORES
    )

    enc = nc.dram_tensor("enc", [SLOC, B, H], F32, kind="ExternalInput")
    vq = nc.dram_tensor("vq", [128, H], F32, kind="ExternalInput")
    eloc = nc.dram_tensor("eloc", [128, NCOLS], F32, kind="ExternalOutput")

    # SBUF (per partition: 6*32 + 4 + 4 + 1 = 201 KiB of ~208 usable)
    tiles = nc.alloc_sbuf_tensor("tiles", [128, SLOTS, GMAX, H], F32)
    vq_sb = nc.alloc_sbuf_tensor("vq_sb", [128, H], F32)
    scratch = nc.alloc_sbuf_tensor("scratch", [128, H], F32)
    partials = nc.alloc_sbuf_tensor("partials", [128, NCOLS], F32)

    def enc_src(t):
        return bass.AP(
            tensor=enc,
            offset=4 * CUMC[t] * B * H,
            ap=[[H, 128], [4 * B * H, GLIST[t]], [1, H]],
        )

    _stack = ExitStack()
    with _stack:
        block = _stack.enter_context(nc.Block(no_gpsimd_drain=True))

        def sem(n):
            return _stack.enter_context(nc.semaphore(n))

        s_vq = sem("s_vq")                              # vq load (+16)
        s_sl = [sem(f"s_sl{j}") for j in range(SLOTS)]  # tile slot loads (+16)
        s_mul = sem("s_mul")       # DVE fused mult+reduce, +1 per tile
        s_out = sem("s_out")       # eloc written (+16)

        # Flush the first ~half of eloc early so only a small store sits on
        # the critical tail.
        SPLIT_T = next(i for i in range(NTILES) if CUMC[i + 1] >= 128)
        SPLIT_C = CUMC[SPLIT_T + 1]

        def tile_dma(eng, t):
            if t >= SLOTS:
                # slot (t%SLOTS) is free once tile t-SLOTS is reduced
                eng.wait_ge(s_mul, t - SLOTS + 1)
            eng.dma_start(
                out=tiles.ap()[:, t % SLOTS, 0:GLIST[t]], in_=enc_src(t)
            ).then_inc(s_sl[t % SLOTS], 16)

        @block.sync
        def _(sp: bass.BassEngine):
            sp.dma_start(out=vq_sb.ap(), in_=vq.ap()).then_inc(s_vq, 16)
            for t in range(1, NTILES, 2):
                tile_dma(sp, t)

        @block.scalar
        def _(act: bass.BassEngine):
            for t in range(0, NTILES, 2):
                tile_dma(act, t)
            act.wait_ge(s_mul, SPLIT_T + 1)
            act.dma_start(out=eloc.ap()[:, 0:SPLIT_C],
                          in_=partials.ap()[:, 0:SPLIT_C]).then_inc(s_out, 16)
            act.wait_ge(s_mul, NTILES)
            act.dma_start(out=eloc.ap()[:, SPLIT_C:NCOLS],
                          in_=partials.ap()[:, SPLIT_C:NCOLS]).then_inc(s_out, 16)
            act.wait_ge(s_out, 32)

        @block.vector
        def _(v: bass.BassEngine):
            v.wait_ge(s_vq, 16)
            for t in range(NTILES):
                v.wait_ge(s_sl[t % SLOTS], 16 * (t // SLOTS + 1))
                for g in range(GLIST[t]):
                    col = CUMC[t] + g
                    op = v.scalar_tensor_tensor(
                        out=scratch.ap(),
                        in0=tiles.ap()[:, t % SLOTS, g],
                        scalar=1.0,
                        in1=vq_sb.ap(),
                        op0=mybir.AluOpType.mult,
                        op1=mybir.AluOpType.mult,
                        accum_out=partials.ap()[:, col : col + 1],
                    )
                    if g == GLIST[t] - 1:
                        op.then_inc(s_mul, 1)

    return nc


def _get_nc():
    if "nc" not in _cache:
        _cache["nc"] = _build()
    return _cache["nc"]


def run(in_maps, trace=False):
    from concourse.bass_utils import run_bass_kernel_spmd

    nc = _get_nc()
    return run_bass_kernel_spmd(
        nc, in_maps, list(range(N_CORES)), trace=trace
    )


def make_in_maps(dec_hidden, enc_outputs, W):
    dec_hidden = np.ascontiguousarray(np.asarray(dec_hidden, dtype=np.float32))
    W = np.ascontiguousarray(np.asarray(W, dtype=np.float32))
    enc_outputs = np.asarray(enc_outputs)
    q = dec_hidden @ W                      # [B, H]
    vq = np.ascontiguousarray(np.tile(q, (4, 1)))  # [128, H]: row p = q[p % 32]
    return [
        {
            "enc": enc_outputs[i * SLOC:(i + 1) * SLOC],
            "vq": vq,
        }
        for i in range(N_CORES)
    ]


def finish(results):
    """Host-side merge: unscramble per-core energies, concat, global softmax.

    eloc[p, col] with p = s4*32 + b holds energy for (b, s_local = 4*col + s4)
    regardless of the tile taper (col = CUMC[t] + g).
    """
    shards = []
    for c in range(N_CORES):
        arr = np.asarray(results[c]["eloc"])            # [128, NCOLS]
        arr = arr.reshape(4, B, NCOLS)                  # [s4, b, col]
        shards.append(arr.transpose(1, 2, 0).reshape(B, SLOC))
    energies = np.concatenate(shards, axis=1)           # [B, S]
    m = energies.max(axis=1, keepdims=True)
    e = np.exp(energies - m, dtype=np.float32)
    return e / e.sum(axis=1, keepdims=True, dtype=np.float32)


def kernel(dec_hidden, enc_outputs, W, bias):
    res = run(make_in_maps(dec_hidden, enc_outputs, W))
    return finish(res.results)



# revision 25
# speedup vs baseline: 1.1851x; 1.0245x over previous
"""Trainium2 Bass kernel for nn_Attention_23003844837848.

energies[b, s] = dec_hidden[b] . (W @ enc_outputs[s, b] + bias)
out = softmax(energies, axis=s)

Rewritten as q = dec_hidden @ W computed on the host (67 MFLOP, trivial;
the dec.bias term is constant per row and cancels inside the softmax), so
the device kernel is a pure 1 GiB streaming dot-product:
  energies[b, s] = q[b] . enc_outputs[s, b]

Distribution: enc_outputs sharded over S across 8 cores (128 MiB/core),
q replicated; each core returns its local energies; the host concatenates
and applies the (tiny, 1 MiB) global softmax.

Layout per core (the key trick vs the previous version): partitions hold
(s4, b) = 4 consecutive s x all 32 b, so partition p's multiplier row
vq[p] = q[p % 32] is the SAME for every tile -- loaded once (512 KiB),
no per-phase broadcast rebuilds (the old kernel burned 16 MiB of HBM on
vrep/W traffic) and no PE transposes (the scrambled [128, 256] energy
block is unscrambled on the host during the gather).

Per tile t: tile[p, g, h] = enc[4*(CUMC[t] + g) + s4, b, h].
A single fused DVE scalar_tensor_tensor per (t, g) column does multiply
AND h-reduction in one streaming pass (accum_out), leaving the ACT engine
free to feed the even-tile DMA queue. Measured on HW: DMA stream ~335 us
busy (~402 GB/s active) when the instance is quiet, DVE ~337 us busy
(256 x 1221 ns fused ops + accumulator reads); total ~363 us vs the
446 us DVE-multiply + ACT-reduce + PE-transpose baseline.
"""

import sys

if "/opt/trn_rl_repo" not in sys.path:
    sys.path.insert(0, "/opt/trn_rl_repo")

from contextlib import ExitStack

import numpy as np

import concourse.bass as bass
from concourse import mybir

S = 8192
B = 32
H = 1024
N_CORES = 8
SLOC = S // N_CORES          # 1024 s per core
# 4 MiB bulk tiles = 1024 DMA descriptors per dma_start: measured sweet
# spot. Smaller tiles pay ~1.9 us of serialized ring overhead per DMA
# (2 MiB x66 -> stream 398 us); bigger ones overflow the HWDGE descriptor
# ring and throttle to ~343 GB/s (8 MiB x21 -> stream 392 us); 4 MiB x35
# streams at ~402 GB/s active (335 us). Small lead tiles start the DVE
# pipeline early; small end tiles shrink the final load->reduce->store.
GMAX = 4                     # s-groups of 4 per tile (max -> 2 MiB tiles)
GLIST = [1, 1, 2] + [GMAX] * 63
CUMC = [0]
for _g in GLIST:
    CUMC.append(CUMC[-1] + _g)
NTILES = len(GLIST)          # 66
NCOLS = CUMC[-1]             # 256 reduction columns (4 s each)
assert NCOLS * 4 == SLOC
SLOTS = 12                   # tile ring slots (even tiles: ACT, odd: SP)
F32 = mybir.dt.float32

_cache = {}


def _build():
    nc = bass.Bass(
        "TRN2", target_bir_lowering=False, debug=False, num_devices=N_CORES
    )

    enc = nc.dram_tensor("enc", [SLOC, B, H], F32, kind="ExternalInput")
    vq = nc.dram_tensor("vq", [128, H], F32, kind="ExternalInput")
    eloc = nc.dram_tensor("eloc", [128, NCOLS], F32, kind="ExternalOutput")

    # SBUF (per partition: 6*32 + 4 + 4 + 1 = 201 KiB of ~208 usable)
    tiles = nc.alloc_sbuf_tensor("tiles", [128, SLOTS, GMAX, H], F32)
    vq_sb = nc.alloc_sbuf_tensor("vq_sb", [128, H], F32)
    scratch = nc.alloc_sbuf_tensor("scratch", [128, H], F32)
    partials = nc.alloc_sbuf_tensor("partials", [128, NCOLS], F32)

    def enc_src(t):
        return bass.AP(
            tensor=enc,
            offset=4 * CUMC[t] * B * H,
            ap=[[H, 128], [4 * B * H, GLIST[t]], [1, H]],
        )

    _stack = ExitStack()
    with _stack:
        block = _stack.enter_context(nc.Block(no_gpsimd_drain=True))

        def sem(n):
            return _stack.enter_context(nc.semaphore(n))

        s_vq = sem("s_vq")                              # vq load (+16)
        s_sl = [sem(f"s_sl{j}") for j in range(SLOTS)]  # tile slot loads (+16)
        s_mul = sem("s_mul")       # DVE fused mult+reduce, +1 per tile
        s_out = sem("s_out")       # eloc written (+16)

        # Flush the first ~half of eloc early so only a small store sits on
        # the critical tail.
        SPLIT_T = next(i for i in range(NTILES) if CUMC[i + 1] >= 128)
        SPLIT_C = CUMC[SPLIT_T + 1]

        def tile_dma(eng, t):
            if t >= SLOTS:
                # slot (t%SLOTS) is free once tile t-SLOTS is reduced
                eng.wait_ge(s_mul, t - SLOTS + 1)
            eng.dma_start(
                out=tiles.ap()[:, t % SLOTS, 0:GLIST[t]], in_=enc_src(t)
            ).then_inc(s_sl[t % SLOTS], 16)

        @block.sync
        def _(sp: bass.BassEngine):
            sp.dma_start(out=vq_sb.ap(), in_=vq.ap()).then_inc(s_vq, 16)
            for t in range(1, NTILES, 2):
                tile_dma(sp, t)

        @block.scalar
        def _(act: bass.BassEngine):
            for t in range(0, NTILES, 2):
                tile_dma(act, t)
            act.wait_ge(s_mul, SPLIT_T + 1)
            act.dma_start(out=eloc.ap()[:, 0:SPLIT_C],
                          in_=partials.ap()[:, 0:SPLIT_C]).then_inc(s_out, 16)
            act.wait_ge(s_mul, NTILES)
            act.dma_start(out=eloc.ap()[:, SPLIT_C:NCOLS],
                          in_=partials.ap()[:, SPLIT_C:NCOLS]).then_inc(s_out, 16)
            act.wait_ge(s_out, 32)

        @block.vector
        def _(v: bass.BassEngine):
            v.wait_ge(s_vq, 16)
            for t in range(NTILES):
                v.wait_ge(s_sl[t % SLOTS], 16 * (t // SLOTS + 1))
                for g in range(GLIST[t]):
                    col = CUMC[t] + g
                    op = v.scalar_tensor_tensor(
                        out=scratch.ap(),
                        in0=tiles.ap()[:, t % SLOTS, g],
                        scalar=1.0,
                        in1=vq_sb.ap(),
                        op0=mybir.AluOpType.mult,
                        op1=mybir.AluOpType.mult,
                        accum_out=partials.ap()[:, col : col + 1],
                    )
                    if g == GLIST[t] - 1:
                        op.then_inc(s_mul, 1)

    return nc


def _get_nc():
    if "nc" not in _cache:
        _cache["nc"] = _build()
    return _cache["nc"]


def run(in_maps, trace=False):
    from concourse.bass_utils import run_bass_kernel_spmd

    nc = _get_nc()
    return run_bass_kernel_spmd(
        nc, in_maps, list(range(N_CORES)), trace=trace
    )


def make_in_maps(dec_hidden, enc_outputs, W):
    dec_hidden = np.ascontiguousarray(np.asarray(dec_hidden, dtype=np.float32))
    W = np.ascontiguousarray(np.asarray(W, dtype=np.float32))
    enc_outputs = np.asarray(enc_outputs)
    q = dec_hidden @ W                      # [B, H]
    vq = np.ascontiguousarray(np.tile(q, (4, 1)))  # [128, H]: row p = q[p % 32]
    return [
        {
            "enc": enc_outputs[i * SLOC:(i + 1) * SLOC],
            "vq": vq,
        }
        for i in range(N_CORES)
    ]


def finish(results):
    """Host-side merge: unscramble per-core energies, concat, global softmax.

    eloc[p, col] with p = s4*32 + b holds energy for (b, s_local = 4*col + s4)
    regardless of the tile taper (col = CUMC[t] + g).
    """
    shards = []
    for c in range(N_CORES):
        arr = np.asarray(results[c]["eloc"])            # [128, NCOLS]
        arr = arr.reshape(4, B, NCOLS)                  # [s4, b, col]
        shards.append(arr.transpose(1, 2, 0).reshape(B, SLOC))
    energies = np.concatenate(shards, axis=1)           # [B, S]
    m = energies.max(axis=1, keepdims=True)
    e = np.exp(energies - m, dtype=np.float32)
    return e / e.sum(axis=1, keepdims=True, dtype=np.float32)


def kernel(dec_hidden, enc_outputs, W, bias):
    res = run(make_in_maps(dec_hidden, enc_outputs, W))
    return finish(res.results)

